# revision 36
# baseline (speedup 1.0000x reference)
"""Self-contained Trainium2 Bass kernel for nn_GCNMagnetModel (3-layer GCN,
N=50000 nodes, E=600000 edges, H=128, 64 graphs, 8 NeuronCores, SPMD 1 NEFF).

v3: ZERO collectives. In this environment any collective_compute pins the
per-execution cost at ~3.5ms regardless of size (measured: one 512-byte
AllGather alone = 3.5ms; the whole rest of the kernel < 0.4ms). So layers 1
and 2 are computed REPLICATED over the full graph on every core (layer 1 is
rank-2: agg((x@W1)dinv) == agg(x dinv)@W1, so its per-edge operand is 2-wide
and ships as a host-laid-out input; layer 2's full table is then locally
buildable), and layer 3 + pooling run per-core on the 8 own graphs. No
inter-core communication at all.

- GCN self-loops are ordinary edges in the lists (table row of dst itself),
  so agg = plain one-hot-matmul segment sum; deg = host bincount + 1 (host
  does integer index work only; rsqrt and all FP-on-values is on device).
- Layers 1/2 aggregate TRANSPOSED (stationary = per-edge operand, moving =
  one-hot) so h feeds the next layer's prepare matmul without PE transposes;
  prepare is fused into the same block iteration and writes the next table
  straight to DRAM per block. Layer 3 aggregates node-major for pooling.
- Per-group streaming of idx/dstloc/x_src/deg_src keeps SBUF small.

kernel(**inputs) -> [64, 41] float32.
"""
import numpy as np
import ml_dtypes
from contextlib import ExitStack

import concourse.tile as tile
import concourse.mybir as mybir
from concourse import bacc
from concourse import library_config
from concourse.bass_utils import run_bass_kernel_spmd

NCORE = 8
P = 128
GPC = 8
H = 128
OC = 41

F32 = mybir.dt.float32
BF16 = mybir.dt.bfloat16
I16 = mybir.dt.int16
AF = mybir.ActivationFunctionType
OP = mybir.AluOpType


def wrap16(v):  # [n] -> [128, n/16]: idx[i%16, i//16] tiled 8x
    a = v.reshape(-1, 16).T
    return np.tile(a, (8, 1)).copy()


def prep(x, edge_index, batch, n_graphs=64):
    N = x.shape[0]
    x = np.asarray(x, np.float32)
    batch = np.asarray(batch)
    src_g, dst_g = np.asarray(edge_index[0]), np.asarray(edge_index[1])

    gstart = np.searchsorted(batch, np.arange(n_graphs), side="left")
    gend = np.searchsorted(batch, np.arange(n_graphs), side="right")
    gsz = gend - gstart

    gblk = np.maximum((gsz + P - 1) // P, 1)
    nblk_core = [int(gblk[k * GPC:(k + 1) * GPC].sum()) for k in range(NCORE)]
    NBLK = max(nblk_core)
    NMAXP = NBLK * P
    NBLKG = NCORE * NBLK

    loc_base = np.zeros(n_graphs, np.int64)
    for g in range(n_graphs):
        if g % GPC == 0:
            loc_base[g] = 0
        else:
            loc_base[g] = loc_base[g - 1] + gblk[g - 1] * P
    node_core = batch // GPC
    node_loc = loc_base[batch] + (np.arange(N) - gstart[batch])
    node_row = node_core * NMAXP + node_loc          # node-order global row
    HALF = 4 * NMAXP
    assert HALF < 32768

    deg = np.bincount(dst_g, minlength=N).astype(np.float32) + 1.0

    # append self loops as ordinary edges
    srcs = np.r_[src_g, np.arange(N)]
    dsts = np.r_[dst_g, np.arange(N)]
    r_src = node_row[srcs]
    r_dst = node_row[dsts]
    x_src = x[srcs]
    deg_src = deg[srcs]

    # ---- global (replicated) layout for layers 1+2 ----
    g_bg = r_dst // P
    g_dl = (r_dst % P).astype(np.float32)
    g_half = (r_src >= HALF).astype(np.int64)
    cntsG = np.zeros((NBLKG, 2), np.int64)
    np.add.at(cntsG, (g_bg, g_half), 1)
    cpG = (cntsG + P - 1) // P
    cpG[:, 0] = np.maximum(cpG[:, 0], 1)          # >=1 chunk so agg PSUM is written
    cpGA, cpGB = cpG[:, 0], cpG[:, 1]
    offGA = np.r_[0, np.cumsum(cpGA)]
    offGB = np.r_[0, np.cumsum(cpGB)]
    NCHGA, NCHGB = int(offGA[-1]), int(offGB[-1])
    NCHG = NCHGA + NCHGB

    orderG = np.lexsort((g_bg, g_half))
    so_bg, so_half = g_bg[orderG], g_half[orderG]
    keyG = so_half * NBLKG + so_bg
    rsG = np.r_[0, np.flatnonzero(np.diff(keyG)) + 1]
    ridG = np.zeros(len(orderG), np.int64)
    ridG[rsG[1:]] = 1
    ridG = np.cumsum(ridG)
    posG = np.arange(len(orderG)) - rsG[ridG]
    so_rsrc, so_dl = r_src[orderG], g_dl[orderG]
    so_x, so_degs = x_src[orderG], deg_src[orderG]
    isAG = so_half == 0
    slotG = np.where(isAG, offGA[so_bg] * P + posG, offGB[so_bg] * P + posG)
    gslot = np.where(isAG, slotG, NCHGA * P + slotG)
    idxG = np.zeros(NCHG * P, np.int16)
    dlG = np.full(NCHG * P, -1.0, np.float32)
    xesG = np.zeros((NCHG * P, 2), np.float32)
    degesG = np.ones(NCHG * P, np.float32)
    idxG[gslot] = np.where(isAG, so_rsrc, so_rsrc - HALF).astype(np.int16)
    dlG[gslot] = so_dl
    xesG[gslot] = so_x
    degesG[gslot] = so_degs

    # ---- per-core layout for layer 3 ----
    e_core = r_dst // NMAXP
    l_blk = (r_dst % NMAXP) // P
    cnt3 = np.zeros((NCORE, NBLK, 2), np.int64)
    np.add.at(cnt3, (e_core, l_blk, g_half), 1)
    cp3 = (cnt3.max(axis=0) + P - 1) // P
    cp3[:, 0] = np.maximum(cp3[:, 0], 1)
    cp3A, cp3B = cp3[:, 0], cp3[:, 1]
    off3A = np.r_[0, np.cumsum(cp3A)]
    off3B = np.r_[0, np.cumsum(cp3B)]
    NCH3A, NCH3B = int(off3A[-1]), int(off3B[-1])
    order3 = np.lexsort((l_blk, g_half, e_core))
    s_core, s_blk, s_half = e_core[order3], l_blk[order3], g_half[order3]
    s_rsrc = r_src[order3]
    s_dl = (r_dst % P)[order3].astype(np.float32)
    key3 = (s_core * 2 + s_half) * NBLK + s_blk
    rs3 = np.r_[0, np.flatnonzero(np.diff(key3)) + 1]
    rid3 = np.zeros(len(order3), np.int64)
    rid3[rs3[1:]] = 1
    rid3 = np.cumsum(rid3)
    pos3 = np.arange(len(order3)) - rs3[rid3]
    idx3A = np.zeros((NCORE, NCH3A * P), np.int16)
    idx3B = np.zeros((NCORE, NCH3B * P), np.int16)
    dl3A = np.full((NCORE, NCH3A * P), -1.0, np.float32)
    dl3B = np.full((NCORE, NCH3B * P), -1.0, np.float32)
    isA3 = s_half == 0
    sl3A = off3A[s_blk[isA3]] * P + pos3[isA3]
    idx3A[s_core[isA3], sl3A] = s_rsrc[isA3].astype(np.int16)
    dl3A[s_core[isA3], sl3A] = s_dl[isA3]
    isB3 = ~isA3
    sl3B = off3B[s_blk[isB3]] * P + pos3[isB3]
    idx3B[s_core[isB3], sl3B] = (s_rsrc[isB3] - HALF).astype(np.int16)
    dl3B[s_core[isB3], sl3B] = s_dl[isB3]

    # ---- degree layouts ----
    degrow = np.ones((NCORE, NMAXP), np.float32)
    degrow[node_core, node_loc] = deg
    degrow_flat = degrow.reshape(-1)                        # [NBLKG*128]
    deg_pm_all = np.ascontiguousarray(
        degrow_flat.reshape(NBLKG, P).T)                    # [128, NBLKG] f32
    degrow_rep = np.tile(degrow_flat[None, :], (P, 1)).astype(ml_dtypes.bfloat16)
    deg_pm = [np.ascontiguousarray(degrow[k].reshape(NBLK, P).T) for k in range(NCORE)]

    # ---- pooling masks (per core) ----
    gonehot = np.zeros((NCORE, NBLK * P, GPC), np.float32)
    gmask = np.zeros((NCORE, GPC, NBLK), np.float32)
    for g in range(n_graphs):
        k, gl = g // GPC, g % GPC
        b0 = loc_base[g] // P
        gmask[k, gl, b0:b0 + gblk[g]] = 1.0
        gonehot[k, loc_base[g]:loc_base[g] + gsz[g], gl] = 1.0

    cores = []
    for k in range(NCORE):
        cores.append(dict(
            idx3A=wrap16(idx3A[k]),
            idx3B=wrap16(idx3B[k]),
            dl3A=np.ascontiguousarray(dl3A[k].reshape(NCH3A, P).T).astype(ml_dtypes.bfloat16),
            dl3B=np.ascontiguousarray(dl3B[k].reshape(NCH3B, P).T).astype(ml_dtypes.bfloat16),
            deg_pm=deg_pm[k],
            gonehot=np.ascontiguousarray(
                gonehot[k].reshape(NBLK, P, GPC).transpose(1, 0, 2)).astype(np.float32),
            gmask=np.tile(gmask[k].reshape(1, GPC * NBLK), (P, 1)).astype(np.float32),
            gvalid=np.tile((gsz[k * GPC:(k + 1) * GPC] > 0).astype(np.float32), (P, 1)),
            cntrep=np.tile(gsz[k * GPC:(k + 1) * GPC].astype(np.float32), (P, 1)),
        ))

    meta = dict(
        NBLK=NBLK, NMAXP=NMAXP, HALF=HALF, NBLKG=NBLKG,
        NCHGA=NCHGA, NCHGB=NCHGB,
        offGA=offGA.astype(int), offGB=offGB.astype(int),
        NCH3A=NCH3A, NCH3B=NCH3B,
        off3A=off3A.astype(int), off3B=off3B.astype(int),
        gsz=gsz, cores=cores,
        idxG=wrap16(idxG),                                   # [128, NCHG*8]
        dlG=np.ascontiguousarray(dlG.reshape(NCHG, P).T).astype(ml_dtypes.bfloat16),
        xesG=np.ascontiguousarray(
            xesG.reshape(NCHG, P, 2).transpose(1, 0, 2)).astype(ml_dtypes.bfloat16),
        degesG=np.ascontiguousarray(degesG.reshape(NCHG, P).T).astype(ml_dtypes.bfloat16),
        deg_pm_all=deg_pm_all,
        degrow_rep=degrow_rep,
    )
    return meta


def build(meta, GBLK=6, GBLK3=6, SINGLE_PACKET=False):
    NBLK, NMAXP, HALF, NBLKG = meta["NBLK"], meta["NMAXP"], meta["HALF"], meta["NBLKG"]
    NCHGA, NCHGB = meta["NCHGA"], meta["NCHGB"]
    offGA, offGB = meta["offGA"], meta["offGB"]
    NCH3A, NCH3B = meta["NCH3A"], meta["NCH3B"]
    off3A, off3B = meta["off3A"], meta["off3B"]
    NCHG = NCHGA + NCHGB
    NTAB = NCORE * NMAXP

    groupsG = []
    for g in range((NBLKG + GBLK - 1) // GBLK):
        b0, b1 = g * GBLK, min((g + 1) * GBLK, NBLKG)
        groupsG.append((b0, b1, int(offGA[b0]), int(offGA[b1]),
                        int(offGB[b0]), int(offGB[b1])))
    GMAXA = max(a1 - a0 for (_, _, a0, a1, _, _) in groupsG)
    GMAXB = max(x1 - x0 for (_, _, _, _, x0, x1) in groupsG)
    groups3 = []
    for g in range((NBLK + GBLK3 - 1) // GBLK3):
        b0, b1 = g * GBLK3, min((g + 1) * GBLK3, NBLK)
        groups3.append((b0, b1, int(off3A[b0]), int(off3A[b1]),
                        int(off3B[b0]), int(off3B[b1])))
    G3MAXA = max(a1 - a0 for (_, _, a0, a1, _, _) in groups3)
    G3MAXB = max(x1 - x0 for (_, _, _, _, x0, x1) in groups3)
    GMAXA = max(GMAXA, G3MAXA)
    GMAXB = max(GMAXB, G3MAXB)
    G3MAXA, G3MAXB = GMAXA, GMAXB

    nc = bacc.Bacc(None, target_bir_lowering=False)

    # ---- IO ----
    idxG_d = nc.dram_tensor("idxG", [128, NCHG * 8], I16, kind="ExternalInput")
    dlG_d = nc.dram_tensor("dlG", [128, NCHG], BF16, kind="ExternalInput")
    xesG_d = nc.dram_tensor("xesG", [128, NCHG, 2], BF16, kind="ExternalInput")
    degesG_d = nc.dram_tensor("degesG", [128, NCHG], BF16, kind="ExternalInput")
    deg_pm_all_d = nc.dram_tensor("deg_pm_all", [128, NBLKG], F32, kind="ExternalInput")
    degrow_rep_d = nc.dram_tensor("degrow_rep", [128, NTAB], BF16, kind="ExternalInput")
    idx3A_d = nc.dram_tensor("idx3A", [128, NCH3A * 8], I16, kind="ExternalInput")
    idx3B_d = nc.dram_tensor("idx3B", [128, NCH3B * 8], I16, kind="ExternalInput")
    dl3A_d = nc.dram_tensor("dl3A", [128, NCH3A], BF16, kind="ExternalInput")
    dl3B_d = nc.dram_tensor("dl3B", [128, NCH3B], BF16, kind="ExternalInput")
    deg_pm_d = nc.dram_tensor("deg_pm", [128, NBLK], F32, kind="ExternalInput")
    colidx_d = nc.dram_tensor("colidx", [128, 128], BF16, kind="ExternalInput")
    ident_d = nc.dram_tensor("ident", [128, 128], F32, kind="ExternalInput")
    W1_d = nc.dram_tensor("W1", [2, H], F32, kind="ExternalInput")
    W2_d = nc.dram_tensor("W2", [H, H], F32, kind="ExternalInput")
    W3_d = nc.dram_tensor("W3", [H, H], F32, kind="ExternalInput")
    Wo_d = nc.dram_tensor("Wo", [H, 2, OC], F32, kind="ExternalInput")
    bo_d = nc.dram_tensor("bo", [GPC, OC], F32, kind="ExternalInput")
    bvec_d = nc.dram_tensor("bvec", [128, 2], F32, kind="ExternalInput")
    brep3_d = nc.dram_tensor("brep3", [128, H], F32, kind="ExternalInput")
    goh_d = nc.dram_tensor("gonehot", [128, NBLK, GPC], F32, kind="ExternalInput")
    gmask_d = nc.dram_tensor("gmask", [128, GPC * NBLK], F32, kind="ExternalInput")
    gvalid_d = nc.dram_tensor("gvalid", [128, GPC], F32, kind="ExternalInput")
    cntrep_d = nc.dram_tensor("cntrep", [128, GPC], F32, kind="ExternalInput")
    out_d = nc.dram_tensor("out", [GPC, OC], F32, kind="ExternalOutput")

    table2_d = nc.dram_tensor("table2", [NTAB, H], BF16)
    table3_d = nc.dram_tensor("table3", [NTAB, H], BF16)

    with tile.TileContext(nc) as tc, ExitStack() as ctx:
        const = ctx.enter_context(tc.tile_pool(name="const", bufs=1))
        resid = ctx.enter_context(tc.tile_pool(name="resid", bufs=1))
        strm = ctx.enter_context(tc.tile_pool(name="strm", bufs=2))
        gap = ctx.enter_context(tc.tile_pool(name="gap", bufs=2))
        gbp = ctx.enter_context(tc.tile_pool(name="gbp", bufs=2))
        ohp = ctx.enter_context(tc.tile_pool(name="ohp", bufs=1))
        wk = ctx.enter_context(tc.tile_pool(name="wk", bufs=3))
        aggps = ctx.enter_context(tc.tile_pool(name="aggps", bufs=3, space="PSUM"))
        prepps = ctx.enter_context(tc.tile_pool(name="prepps", bufs=2, space="PSUM"))
        tps = ctx.enter_context(tc.tile_pool(name="tps", bufs=2, space="PSUM"))
        poolps = ctx.enter_context(tc.tile_pool(name="poolps", bufs=1, space="PSUM"))

        nc.gpsimd.load_library(library_config.mlp)

        def load_const(dram, shape, dt):
            t = const.tile(shape, dt, tag=dram.name)
            nc.sync.dma_start(t[:], dram[:])
            return t

        dlG_t = load_const(dlG_d, [128, NCHG], BF16)
        idx3A_t = load_const(idx3A_d, [128, NCH3A * 8], I16)
        idx3B_t = load_const(idx3B_d, [128, NCH3B * 8], I16)
        dl3A_t = load_const(dl3A_d, [128, NCH3A], BF16)
        dl3B_t = load_const(dl3B_d, [128, NCH3B], BF16)
        deg_pm_all_t = load_const(deg_pm_all_d, [128, NBLKG], F32)
        deg_pm_t = load_const(deg_pm_d, [128, NBLK], F32)
        colidx_t = load_const(colidx_d, [128, 128], BF16)
        ident_t = load_const(ident_d, [128, 128], F32)
        W1_t = load_const(W1_d, [2, H], F32)
        W2_t = load_const(W2_d, [H, H], F32)
        W3_t = load_const(W3_d, [H, H], F32)
        Wo_t = load_const(Wo_d, [H, 2, OC], F32)
        bo_t = load_const(bo_d, [GPC, OC], F32)
        bvec_t = load_const(bvec_d, [128, 2], F32)
        brep3_t = load_const(brep3_d, [128, H], F32)
        goh_t = load_const(goh_d, [128, NBLK, GPC], F32)
        gmask_t = load_const(gmask_d, [128, GPC * NBLK], F32)
        gvalid_t = load_const(gvalid_d, [128, GPC], F32)
        cntrep_t = load_const(cntrep_d, [128, GPC], F32)

        # dinv for prepare scaling (partition-major over all global blocks)
        dinv_all = resid.tile([128, NBLKG], F32, tag="dinv_all")
        rec_all = resid.tile([128, NBLKG], F32, tag="rec_all")
        nc.vector.reciprocal(rec_all[:], deg_pm_all_t[:])
        nc.scalar.activation(dinv_all[:], rec_all[:], AF.Sqrt)
        dinv_pm = resid.tile([128, NBLK], F32, tag="dinv_pm")
        rec_pm = resid.tile([128, NBLK], F32, tag="rec_pm")
        nc.vector.reciprocal(rec_pm[:], deg_pm_t[:])
        nc.scalar.activation(dinv_pm[:], rec_pm[:], AF.Sqrt)

        W1b = const.tile([2, H], BF16, tag="W1b")
        nc.vector.tensor_copy(W1b[:], W1_t[:])
        W2b = const.tile([H, H], BF16, tag="W2b")
        nc.vector.tensor_copy(W2b[:], W2_t[:])
        W3b = const.tile([H, H], BF16, tag="W3b")
        nc.vector.tensor_copy(W3b[:], W3_t[:])

        # per-edge-slot u = x_src * rsqrt(deg_src): built chunkwise, resident
        ues = resid.tile([128, NCHG, 2], BF16, tag="ues")
        UCH = (NCHG + 7) // 8
        for u0 in range(0, NCHG, UCH):
            u1 = min(u0 + UCH, NCHG)
            un = u1 - u0
            xch = strm.tile([128, UCH, 2], BF16, tag="xch")
            nc.sync.dma_start(xch[:, :un, :], xesG_d[:, u0:u1, :])
            dch = strm.tile([128, UCH], BF16, tag="dch")
            nc.sync.dma_start(dch[:, :un], degesG_d[:, u0:u1])
            rch = strm.tile([128, UCH], BF16, tag="rch")
            with nc.allow_low_precision(reason="deg exact in bf16"):
                nc.vector.reciprocal(rch[:, :un], dch[:, :un])
            sch = strm.tile([128, UCH], BF16, tag="sch")
            nc.scalar.activation(sch[:, :un], rch[:, :un], AF.Sqrt)
            nc.vector.tensor_tensor(
                ues[:, u0:u1, :], xch[:, :un, :],
                sch[:, :un, None].broadcast_to((128, un, 2)), OP.mult)

        # dinv in row layout for the transposed epilogues: computed once into a
        # DRAM scratch (keeps Sqrt off the ACT engine inside the loops)
        dinvrow_d = nc.dram_tensor("dinvrow", [128, NTAB], BF16)
        DCH = NMAXP // 2
        for k0 in range(0, NTAB, DCH):
            dgc = strm.tile([128, DCH], BF16, tag="dgc")
            nc.sync.dma_start(dgc[:], degrow_rep_d[:, k0:k0 + DCH])
            rgc = strm.tile([128, DCH], BF16, tag="rgc")
            with nc.allow_low_precision(reason="deg exact in bf16"):
                nc.vector.reciprocal(rgc[:], dgc[:])
            drc = strm.tile([128, DCH], BF16, tag="drc")
            nc.scalar.activation(drc[:], rgc[:], AF.Sqrt)
            nc.sync.dma_start(dinvrow_d[:, k0:k0 + DCH], drc[:])

        meanp = poolps.tile([128, GPC], F32, tag="meanp")
        pmax = resid.tile([128, NBLK], F32, tag="pmax")

        def build_oh(dlA_ap, dlB_ap, na, nb, gma, gmb):
            oh = ohp.tile([128, gma + gmb, 128], BF16, tag="oh")
            if na:
                cb = colidx_t[:, None, :].broadcast_to((128, na, 128))
                db = dlA_ap[:, :, None].broadcast_to((128, na, 128))
                nc.vector.tensor_tensor(oh[:, :na, :], cb, db, OP.is_equal)
            if nb:
                cb = colidx_t[:, None, :].broadcast_to((128, nb, 128))
                db = dlB_ap[:, :, None].broadcast_to((128, nb, 128))
                nc.vector.tensor_tensor(oh[:, na:na + nb, :], cb, db, OP.is_equal)
            return oh

        def dinvrep_group(b0, b1):
            n = (b1 - b0) * 128
            dr = strm.tile([128, GBLK * 128], BF16, tag="dr")
            nc.sync.dma_start(dr[:, :n], dinvrow_d[:, b0 * 128:b1 * 128])
            return dr

        # ================= Layer 1 (replicated, rank-2) + table2 ============
        for (b0, b1, a0, a1, x0, x1) in groupsG:
            na, nb = a1 - a0, x1 - x0
            oh = build_oh(dlG_t[:, a0:a1] if na else None,
                          dlG_t[:, NCHGA + x0:NCHGA + x1] if nb else None,
                          na, nb, GMAXA, GMAXB)
            dr = dinvrep_group(b0, b1)
            for b in range(b0, b1):
                aggUT_full = aggps.tile([128, 128], F32, tag="agg")
                aggUT = aggUT_full[0:2, :]
                cks = [(c, c) for c in range(int(offGA[b]) - a0, int(offGA[b + 1]) - a0)]
                cks += [(NCHGA - a0 + x0 + c, na + c)
                        for c in range(int(offGB[b]) - x0, int(offGB[b + 1]) - x0)]
                for i, (cu, c) in enumerate(cks):
                    nc.tensor.matmul(aggUT, ues[:, a0 + cu, :], oh[:, c, :],
                                     start=(i == 0), stop=(i == len(cks) - 1))
                cU = wk.tile([2, 128], BF16, tag="cU")
                nc.scalar.copy(cU[:], aggUT)
                hpreT = tps.tile([H, 128], F32, tag="tp")
                nc.tensor.matmul(hpreT[:], W1b[:], cU[:], start=True, stop=True)
                e1 = wk.tile([128, 128], BF16, tag="e1")
                nc.vector.tensor_tensor(
                    e1[:], hpreT[:], dr[:, (b - b0) * 128:(b - b0 + 1) * 128], OP.mult)
                hT = wk.tile([128, 128], BF16, tag="hT")
                nc.scalar.activation(hT[:], e1[:], AF.Tanh, bias=bvec_t[:, 0:1])
                pp = prepps.tile([128, H], F32, tag="pp")
                nc.tensor.matmul(pp[:], hT[:], W2b[:], start=True, stop=True)
                t12 = wk.tile([128, H], BF16, tag="t12")
                nc.vector.tensor_scalar(
                    t12[:], pp[:], dinv_all[:, b:b + 1], None, OP.mult)
                nc.sync.dma_start(table2_d[b * 128:(b + 1) * 128, :], t12[:])

        # ================= Layer 2 (replicated) + table3 ====================
        for (b0, b1, a0, a1, x0, x1) in groupsG:
            na, nb = a1 - a0, x1 - x0
            gA = gB = None
            if na:
                ixA = strm.tile([128, GMAXA * 8], I16, tag="ixA")
                nc.sync.dma_start(ixA[:, :na * 8], idxG_d[:, a0 * 8:a1 * 8])
                gA = gap.tile([128, GMAXA, H], BF16, tag="gA")
                nc.gpsimd.dma_gather(
                    gA[:, :na, :], table2_d[0:HALF, :],
                    ixA[:, :na * 8], na * 128, na * 128, H,
                    single_packet=SINGLE_PACKET)
            if nb:
                ixB = strm.tile([128, GMAXB * 8], I16, tag="ixB")
                nc.sync.dma_start(ixB[:, :nb * 8], idxG_d[:, (NCHGA + x0) * 8:(NCHGA + x1) * 8])
                gB = gbp.tile([128, GMAXB, H], BF16, tag="gB")
                nc.gpsimd.dma_gather(
                    gB[:, :nb, :], table2_d[HALF:, :],
                    ixB[:, :nb * 8], nb * 128, nb * 128, H,
                    single_packet=SINGLE_PACKET)
            oh = build_oh(dlG_t[:, a0:a1] if na else None,
                          dlG_t[:, NCHGA + x0:NCHGA + x1] if nb else None,
                          na, nb, GMAXA, GMAXB)
            dr = dinvrep_group(b0, b1)
            for b in range(b0, b1):
                aggT = aggps.tile([128, 128], F32, tag="agg")
                mms = [(gA, c, c) for c in range(int(offGA[b]) - a0, int(offGA[b + 1]) - a0)]
                mms += [(gB, c, na + c) for c in range(int(offGB[b]) - x0, int(offGB[b + 1]) - x0)]
                for i, (gt, c, co) in enumerate(mms):
                    nc.tensor.matmul(aggT[:], gt[:, c, :], oh[:, co, :],
                                     start=(i == 0), stop=(i == len(mms) - 1))
                e1 = wk.tile([128, 128], BF16, tag="e1")
                nc.vector.tensor_tensor(
                    e1[:], aggT[:], dr[:, (b - b0) * 128:(b - b0 + 1) * 128], OP.mult)
                hT = wk.tile([128, 128], BF16, tag="hT")
                nc.scalar.activation(hT[:], e1[:], AF.Tanh, bias=bvec_t[:, 1:2])
                pp = prepps.tile([128, H], F32, tag="pp")
                nc.tensor.matmul(pp[:], hT[:], W3b[:], start=True, stop=True)
                t13 = wk.tile([128, H], BF16, tag="t13")
                nc.vector.tensor_scalar(
                    t13[:], pp[:], dinv_all[:, b:b + 1], None, OP.mult)
                nc.sync.dma_start(table3_d[b * 128:(b + 1) * 128, :], t13[:])

        # ================= Layer 3 (per-core, node-major) + pooling =========
        for (b0, b1, a0, a1, x0, x1) in groups3:
            na, nb = a1 - a0, x1 - x0
            gA = gB = None
            if na:
                gA = gap.tile([128, G3MAXA, H], BF16, tag="gA")
                nc.gpsimd.dma_gather(
                    gA[:, :na, :], table3_d[0:HALF, :],
                    idx3A_t[:, a0 * 8:a1 * 8], na * 128, na * 128, H,
                    single_packet=SINGLE_PACKET)
            if nb:
                gB = gbp.tile([128, G3MAXB, H], BF16, tag="gB")
                nc.gpsimd.dma_gather(
                    gB[:, :nb, :], table3_d[HALF:, :],
                    idx3B_t[:, x0 * 8:x1 * 8], nb * 128, nb * 128, H,
                    single_packet=SINGLE_PACKET)
            oh = build_oh(dl3A_t[:, a0:a1] if na else None,
                          dl3B_t[:, x0:x1] if nb else None, na, nb, G3MAXA, G3MAXB)
            for b in range(b0, b1):
                agg = aggps.tile([128, H], F32, tag="agg")
                mms = [(gA, c, c) for c in range(int(off3A[b]) - a0, int(off3A[b + 1]) - a0)]
                mms += [(gB, c, na + c) for c in range(int(off3B[b]) - x0, int(off3B[b + 1]) - x0)]
                for i, (gt, c, co) in enumerate(mms):
                    nc.tensor.matmul(agg[:], oh[:, co, :], gt[:, c, :],
                                     start=(i == 0), stop=(i == len(mms) - 1))
                e2 = wk.tile([128, H], F32, tag="e2")
                nc.vector.scalar_tensor_tensor(
                    e2[:], agg[:], dinv_pm[:, b:b + 1], brep3_t[:], OP.mult, OP.add)
                hblk = wk.tile([128, H], F32, tag="hblk")
                nc.scalar.activation(hblk[:], e2[:], AF.Tanh)
                nc.tensor.matmul(meanp[:], hblk[:], goh_t[:, b, :],
                                 start=(b == 0), stop=(b == NBLK - 1))
                tp = tps.tile([128, H], F32, tag="tp")
                nc.tensor.transpose(tp[:], hblk[:], ident_t[:])
                nc.vector.tensor_reduce(
                    pmax[:, b:b + 1], tp[:], mybir.AxisListType.X, OP.max)

        # ---- pooling tail + head ----
        p2 = resid.tile([128, NBLK], F32, tag="p2")
        nc.vector.tensor_scalar(p2[:], pmax[:], 2.0, None, OP.add)
        mg = wk.tile([128, GPC, NBLK], F32, tag="mg")
        nc.vector.tensor_tensor(
            mg[:], p2[:, None, :].broadcast_to((128, GPC, NBLK)),
            gmask_t[:].rearrange("p (g b) -> p g b", g=GPC), OP.mult)
        mxT = resid.tile([128, GPC], F32, tag="mxT")
        nc.vector.tensor_reduce(
            mxT[:, :, None], mg[:], mybir.AxisListType.X, OP.max)
        mxT2 = resid.tile([128, GPC], F32, tag="mxT2")
        nc.vector.scalar_tensor_tensor(
            mxT2[:], mxT[:], -2.0, gvalid_t[:], OP.add, OP.mult)

        cmax = wk.tile([128, GPC], F32, tag="cmax")
        nc.vector.tensor_scalar(cmax[:], cntrep_t[:], 1.0, None, OP.max)
        crec = wk.tile([128, GPC], F32, tag="crec")
        nc.vector.reciprocal(crec[:], cmax[:])
        meanT = wk.tile([128, GPC], F32, tag="meanT")
        nc.vector.tensor_tensor(meanT[:], meanp[:], crec[:], OP.mult)

        headp_full = prepps.tile([128, H], F32, tag="pp")
        headp = headp_full[0:GPC, 0:OC]
        nc.tensor.matmul(headp, mxT2[:], Wo_t[:, 0, :], start=True, stop=False)
        nc.tensor.matmul(headp, meanT[:], Wo_t[:, 1, :], start=False, stop=True)
        hsum = wk.tile([GPC, OC], F32, tag="hsum")
        nc.vector.tensor_tensor(hsum[:], headp, bo_t[:], OP.add)
        ofin = wk.tile([GPC, OC], F32, tag="ofin")
        nc.scalar.activation(ofin[:], hsum[:], AF.Tanh)
        nc.sync.dma_start(out_d[:], ofin[:])

    nc.compile()
    return nc


def make_in_maps(meta, inputs):
    colidx = np.tile(np.arange(128, dtype=np.float32), (128, 1)).astype(ml_dtypes.bfloat16)
    bvec = np.stack([np.asarray(inputs["b1"], np.float32),
                     np.asarray(inputs["b2"], np.float32)], axis=1)
    brep3 = np.tile(np.asarray(inputs["b3"], np.float32), (P, 1))
    bo_t = np.tile(np.asarray(inputs["bo"], np.float32), (GPC, 1))
    Wo = np.asarray(inputs["Wo"], np.float32)
    shared = {
        "idxG": meta["idxG"], "dlG": meta["dlG"],
        "xesG": meta["xesG"], "degesG": meta["degesG"],
        "deg_pm_all": meta["deg_pm_all"], "degrow_rep": meta["degrow_rep"],
        "colidx": colidx, "ident": np.eye(128, dtype=np.float32),
        "W1": np.asarray(inputs["W1"], np.float32),
        "W2": np.asarray(inputs["W2"], np.float32),
        "W3": np.asarray(inputs["W3"], np.float32),
        "Wo": np.ascontiguousarray(np.stack([Wo[:H], Wo[H:]], axis=1)),
        "bo": bo_t, "bvec": bvec, "brep3": brep3,
    }
    maps = []
    for c in meta["cores"]:
        m = dict(shared)
        m.update({
            "idx3A": c["idx3A"], "idx3B": c["idx3B"],
            "dl3A": c["dl3A"], "dl3B": c["dl3B"],
            "deg_pm": c["deg_pm"],
            "gonehot": c["gonehot"], "gmask": c["gmask"],
            "gvalid": c["gvalid"], "cntrep": c["cntrep"],
        })
        maps.append(m)
    return maps


_CACHE = {}


def kernel(x, edge_index, batch, W1, b1, W2, b2, W3, b3, Wo, bo):
    x = np.asarray(x, np.float32)
    edge_index = np.asarray(edge_index)
    batch = np.asarray(batch)
    meta = prep(x, edge_index, batch, 64)
    key = (meta["NBLK"], meta["NCHGA"], meta["NCHGB"], meta["NCH3A"], meta["NCH3B"])
    if key not in _CACHE:
        _CACHE[key] = build(meta)
    nc = _CACHE[key]
    inputs = dict(W1=W1, b1=b1, W2=W2, b2=b2, W3=W3, b3=b3, Wo=Wo, bo=bo)
    in_maps = make_in_maps(meta, inputs)
    res = run_bass_kernel_spmd(nc, in_maps, core_ids=list(range(8)), trace=False)
    out = np.concatenate([res.results[k]["out"] for k in range(8)], 0)
    return np.ascontiguousarray(out, dtype=np.float32)
